# revision 34
# baseline (speedup 1.0000x reference)
"""Trainium2 Bass kernel for nn_Attention (S=2048, B=2, D=1024, H=16, C=64).

Tensor-parallel over heads across 8 NeuronCores (2 heads/core), fully
interleaved wavefront:
  - All static inputs are pre-cast to bf16 on the host, so every load is
    a plain (non-casting) DMA and queue assignment is free.
  - p1 (projections+norm+rope, 8 chunks of 512 tokens): Wq/Wk pre-scaled
    by the RMSNorm weights on host; sumsq of the raw projection recovered
    via matmul against 1/w^2 selector columns; rstd computed as
    exp(-0.5*ln(ms/C+eps)) on ACT -- Ln and Exp share one activation
    table with the softmax Exp, so the kernel never swaps ACT tables;
    rstd returns as a stride-0 broadcast DMA and is applied as the last
    multiply (rope commutes with the per-token scale); the rope
    pair-swap is a PE permutation matmul.  The elementwise chain is bf16
    and split DVE/GpSimd: chunks 0-3 run fully on DVE (GpSimd is kept
    clear so the warmup collective can block it harmlessly), chunks 4-7
    use GpSimd for the adds/final muls.
  - p2 (attention, 8 query chunks of 512): scores transposed [keys, q]
    per head with K=64 contraction on PE quadrants; one [128,1024] exp
    per key block covers both heads; attn@v accumulates in PSUM with an
    appended ones column so the softmax denominator falls out.  The
    denominator reciprocal runs on DVE straight out of PSUM and returns
    as a stride-0 broadcast DMA (sync+vector rings), no reshape hops.
  - The AllToAll re-shard runs at chunk granularity (8 collectives,
    destination core = (token//64) % 8), each fired right after its
    chunk.  All collectives issue from GpSimd (NRT needs straight-line
    collective order); a FULL-SIZE warmup AllToAll fires at kernel start
    so the ~70us one-time setup for the 128KB transfer shape overlaps
    p1 instead of stalling the first real collective.  p3 out-projection
    runs per received eighth with its DMAs on the vector ring so a slow
    collective cannot head-of-line-block the sync ring.
  - p1 pieces and p3 eighths drain from an interleave queue between p2
    key blocks so the PE stays continuously busy.
"""

import sys

if "/opt/trn_rl_repo" not in sys.path:
    sys.path.insert(0, "/opt/trn_rl_repo")

import numpy as np
import ml_dtypes
import concourse.bass as bass
import concourse.hw_specs as _hw_specs
from concourse import bacc, tile, mybir
from concourse.bass_utils import run_bass_kernel_spmd
from concourse.masks import make_identity

# The act-table selector is first-fit per function, which lands Exp in
# 'exp_and_others' and Ln in 'natural_log' and then thrashes 1.3us table
# loads between them.  'natural_log_exp_and_others' genuinely contains
# every ACT function this kernel uses (Exp, Ln, Square), so mask those
# functions out of the other tables; the emitted act_func_set_id then
# points at the real combined table in act_info.json.
_ORIG_ACT_TABLES = _hw_specs.get_activation_tables


def _combined_act_tables(arch):
    AFT = mybir.ActivationFunctionType
    keep = {AFT.Exp, AFT.Ln, AFT.Square}
    out = {}
    for name, funcs in _ORIG_ACT_TABLES(arch).items():
        if name != "natural_log_exp_and_others":
            funcs = set(funcs) - keep
        out[name] = set(funcs)
    return out


bacc.get_activation_tables = _combined_act_tables

S, B, D, H, C = 2048, 2, 1024, 16, 64
EPS = 1e-6
NCORES = 8
T = S * B                  # 4096 tokens, batch-major: t = b*S + s
LH = H // NCORES           # 2 local heads
LC = LH * C                # 128 local head columns
TCH = 512                  # p1/p2 token chunk
NCH = T // TCH             # 8
NJT = S // 128             # 16 key blocks per batch
TOK_OUT = T // NCORES      # 512 output tokens per core

F32 = mybir.dt.float32
F32R = mybir.dt.float32r
BF16 = mybir.dt.bfloat16
AF = mybir.ActivationFunctionType

_CACHE = {}
LAST_RESULTS = None


def _build():
    nc = bacc.Bacc("TRN2", target_bir_lowering=False, debug=False,
                   num_devices=NCORES)
    xT = nc.dram_tensor("xT", [D, T], BF16, kind="ExternalInput")
    wqkv = nc.dram_tensor("wqkv", [D, 3 * LC], BF16, kind="ExternalInput")
    wo = nc.dram_tensor("wo", [H * C, D], BF16, kind="ExternalInput")
    ropeAB = nc.dram_tensor("ropeAB", [LC, 2 * S], BF16,
                            kind="ExternalInput")
    winv = nc.dram_tensor("winv", [128, 8], BF16, kind="ExternalInput")
    perm = nc.dram_tensor("perm", [128, 128], BF16, kind="ExternalInput")
    bseld = nc.dram_tensor("bsel", [4, 256], BF16, kind="ExternalInput")
    out = nc.dram_tensor("out", [TOK_OUT, D], F32, kind="ExternalOutput")

    xT4 = xT.rearrange("(a p) t -> p a t", p=128)        # [128, 8, T]
    wqkv4 = wqkv.rearrange("(a p) c -> p a c", p=128)    # [128, 8, 3*LC]
    wo4 = wo.rearrange("(a p) n -> p a n", p=128)        # [128, 8, D]
    rope4 = ropeAB.rearrange("p (j t) -> p j t", j=2)    # [128, 2, S]

    with tile.TileContext(nc) as tc:
        with (
            tc.tile_pool(name="singles", bufs=1) as singles,
            tc.tile_pool(name="xtp", bufs=5) as xtp,
            tc.tile_pool(name="workp", bufs=2) as workp,
            tc.tile_pool(name="ps1", bufs=2, space="PSUM") as ps1p,
            tc.tile_pool(name="pss", bufs=2, space="PSUM") as pssp,
            tc.tile_pool(name="pos", bufs=2, space="PSUM") as posp,
            tc.tile_pool(name="dram", bufs=1, space="DRAM") as dram,
        ):
            # ---- constants ----
            ident = singles.tile([128, 128], BF16)
            make_identity(nc, ident)
            winv_sb = singles.tile([128, 8], BF16)
            nc.gpsimd.dma_start(out=winv_sb, in_=winv[:, :])
            permb = singles.tile([128, 128], BF16)
            nc.gpsimd.dma_start(out=permb, in_=perm[:, :])
            eps128 = singles.tile([128, 1], F32)
            nc.vector.memset(eps128, EPS)
            wseed = singles.tile([128, 8, 128], BF16)
            nc.vector.memset(wseed, 0.0)
            # broadcast selectors: bq rows <- rstd rows 0/1, bk <- 2/3,
            # dnb <- row 0 of its operand (bsel[0:1, 0:64] is all-ones)
            bsel = singles.tile([4, 256], BF16)
            nc.gpsimd.dma_start(out=bsel, in_=bseld[:, :])

            # ---- weights / rope (wo deferred: only p3 needs it) ----
            wqkv_sb = singles.tile([128, 8, 3 * LC], BF16)
            nc.sync.dma_start(out=wqkv_sb, in_=wqkv4)
            rope_sb = singles.tile([128, 2, S], BF16)
            wo_sb = singles.tile([128, 8, D], BF16)

            # ---- persistent activations ----
            qT_sb = singles.tile([128, T], BF16)   # [2 heads x 64c, t]
            k_sb = singles.tile([128, T], BF16)
            v_sb = singles.tile([128, T // 128, LH, C + 1], BF16)
            onescol = singles.tile([128, T // 128, LH, 1], F32)
            nc.vector.memset(onescol, 1.0)
            nc.vector.tensor_copy(v_sb[:, :, :, C:C + 1], onescol)

            # chunk PAIRS share one AllToAll: [1024 rows, 128 tokens]
            NPAIR = NCH // 2
            wbin = dram.tile([NCORES * 128, 128], BF16, tag="wbin",
                             name="wbin")
            wbout = dram.tile([NCORES * 128, 128], BF16, tag="wbout",
                              name="wbout")
            bins = [dram.tile([NCORES * 128, 128], BF16, tag=f"bin{q}",
                              name=f"bin{q}") for q in range(NPAIR)]
            bouts = [dram.tile([NCORES * 128, 128], BF16, tag=f"bout{q}",
                               name=f"bout{q}") for q in range(NPAIR)]

            xts = {}
            st = {ch: {} for ch in range(NCH)}

            def load_xt(ch):
                xt = xtp.tile([128, 8, TCH], BF16, tag="xt", name=f"xt{ch}")
                eng = nc.sync if ch % 2 == 0 else nc.scalar
                sl = slice(ch * TCH, (ch + 1) * TCH)
                if ch < 2:
                    # split: the a<4 half lands first so the first
                    # projection matmuls start ~8us earlier
                    eng.dma_start(out=xt[:, 0:4, :], in_=xT4[:, 0:4, sl])
                    eng.dma_start(out=xt[:, 4:8, :], in_=xT4[:, 4:8, sl])
                else:
                    eng.dma_start(out=xt, in_=xT4[:, :, sl])
                xts[ch] = xt

            # ---- p1 pieces (ACT: squares for ch<4, ln+exp rstd) ----
            def p1_q(ch):
                if ch + 3 < NCH:
                    load_xt(ch + 3)
                psq = ps1p.tile([128, TCH], F32, tag="ps1", name=f"psq{ch}")
                for a in range(8):
                    nc.tensor.matmul(psq, wqkv_sb[:, a, 0:LC],
                                     xts[ch][:, a, :],
                                     start=(a == 0), stop=(a == 7))
                qraw = workp.tile([128, TCH], BF16, tag="qraw", bufs=6,
                                  name=f"qraw{ch}")
                nc.vector.tensor_copy(qraw, psq)
                sq2 = workp.tile([128, TCH], BF16, tag="sq2",
                                 name=f"sq2_{ch}")
                if ch < 4:
                    nc.scalar.activation(sq2, psq, AF.Square)
                else:
                    nc.vector.tensor_mul(sq2, qraw, qraw)
                st[ch]["qraw"], st[ch]["sq2"] = qraw, sq2

            def p1_k(ch):
                psk = ps1p.tile([128, TCH], F32, tag="ps1", name=f"psk{ch}")
                for a in range(8):
                    nc.tensor.matmul(psk, wqkv_sb[:, a, LC:2 * LC],
                                     xts[ch][:, a, :],
                                     start=(a == 0), stop=(a == 7))
                kraw = workp.tile([128, TCH], BF16, tag="kraw", bufs=6,
                                  name=f"kraw{ch}")
                nc.vector.tensor_copy(kraw, psk)
                sqk2 = workp.tile([128, TCH], BF16, tag="sqk2",
                                  name=f"sqk2_{ch}")
                if ch < 4:
                    nc.scalar.activation(sqk2, psk, AF.Square)
                else:
                    nc.vector.tensor_mul(sqk2, kraw, kraw)
                st[ch]["kraw"], st[ch]["sqk2"] = kraw, sqk2

            def p1_v(ch):
                t0 = ch * TCH
                psv = ps1p.tile([128, TCH], F32, tag="ps1", name=f"psv{ch}")
                for a in range(8):
                    nc.tensor.matmul(psv, wqkv_sb[:, a, 2 * LC:3 * LC],
                                     xts[ch][:, a, :],
                                     start=(a == 0), stop=(a == 7))
                vt = workp.tile([128, TCH], BF16, tag="vt", name=f"vt{ch}")
                nc.vector.tensor_copy(vt, psv)
                ptv = ps1p.tile([128, 4, 128], BF16, tag="ps1",
                                name=f"ptv{ch}")
                for s5 in range(TCH // 128):
                    nc.tensor.transpose(ptv[:, s5, :],
                                        vt[:, s5 * 128:(s5 + 1) * 128],
                                        ident)
                blk0 = t0 // 128
                nc.vector.tensor_copy(
                    v_sb[:, blk0:blk0 + 4, :, 0:C],
                    ptv.rearrange("p f (l c) -> p f l c", l=LH))

            def p1_ms(ch):
                # sumsq rows 0:2 = q heads, 2:4 = k heads (one bank)
                ms = ps1p.tile([4, TCH], F32, tag="ps1", name=f"ms{ch}")
                nc.tensor.matmul(ms, winv_sb[:, 0:4], st[ch]["sq2"],
                                 start=True, stop=False)
                nc.tensor.matmul(ms, winv_sb[:, 4:8], st[ch]["sqk2"],
                                 start=False, stop=True)
                st[ch]["ms"] = ms

            def p1_rstd(ch):
                # rstd = exp(-0.5 * ln(ms/C + eps)); Ln+Exp live in the
                # same ACT table as the softmax Exp -> no table swaps.
                lnv = workp.tile([4, TCH], F32, tag="lnv", bufs=4,
                                 name=f"lnv{ch}")
                nc.scalar.activation(lnv, st[ch]["ms"], AF.Ln,
                                     bias=eps128[0:4, :], scale=1.0 / C)
                rstd = workp.tile([4, TCH], BF16, tag="rstd", bufs=4,
                                  name=f"rstd{ch}")
                nc.scalar.activation(rstd, lnv, AF.Exp, bias=0.0,
                                     scale=-0.5)
                st[ch]["rstd"] = rstd

            def p1_fin(ch):
                t0 = ch * TCH
                t0s = t0 % S
                # per-token rstd broadcast via K=4 selector matmuls (bf16)
                for which, raw, c0, dst in (
                        ("q", st[ch]["qraw"], 0, qT_sb),
                        ("k", st[ch]["kraw"], 128, k_sb)):
                    bw = ps1p.tile([128, TCH], F32, tag="ps1",
                                   name=f"bw{which}{ch}")
                    nc.tensor.matmul(bw, bsel[:, c0:c0 + 128],
                                     st[ch]["rstd"], start=True, stop=True)
                    t1 = workp.tile([128, TCH], BF16, tag="t1",
                                    name=f"t1{which}{ch}")
                    nc.vector.tensor_mul(t1, rope_sb[:, 0, t0s:t0s + TCH],
                                         raw)
                    rot = ps1p.tile([128, TCH], F32, tag="ps1",
                                    name=f"rot{which}{ch}")
                    nc.tensor.matmul(rot, permb, raw, start=True, stop=True)
                    t2 = workp.tile([128, TCH], BF16, tag="t2",
                                    name=f"t2{which}{ch}")
                    nc.vector.tensor_mul(t2, rope_sb[:, 1, t0s:t0s + TCH],
                                         rot)
                    t3 = workp.tile([128, TCH], BF16, tag="t3",
                                    name=f"t3{which}{ch}")
                    nc.gpsimd.tensor_add(t3, t1, t2)
                    nc.vector.tensor_mul(dst[:, t0:t0 + TCH], t3, bw)

            # =============== phase-2 =====================================
            pos_tiles = {}

            p2_exs = {}

            def p2_part(c, r0, r1, interleave, pop_every=False):
                b = c // 4
                q0 = c * TCH
                if c not in pos_tiles:
                    pos_tiles[c] = [posp.tile([C + 1, TCH], F32, tag="pos",
                                              name=f"pos{c}_{lh}")
                                    for lh in range(LH)]
                    p2_exs[c] = {}
                exs = p2_exs[c]

                def scores_exp(jt):
                    j0 = b * S + jt * 128
                    pss = pssp.tile([128, LH, TCH], F32, tag="pss",
                                    name=f"pss{c}_{jt}")
                    for lh in range(LH):
                        nc.tensor.matmul(
                            pss[:, lh, :],
                            k_sb[64 * lh:64 * lh + 64, j0:j0 + 128],
                            qT_sb[64 * lh:64 * lh + 64, q0:q0 + TCH],
                            start=True, stop=True)
                    ex = workp.tile([128, LH, TCH], BF16, tag="ex", bufs=11,
                                    name=f"ex{c}_{jt}")
                    nc.scalar.activation(ex, pss, AF.Exp, bias=0.0,
                                         scale=0.125)
                    exs[jt] = ex

                def attnv(jt):
                    j0 = b * S + jt * 128
                    for lh in range(LH):
                        nc.tensor.matmul(
                            pos_tiles[c][lh],
                            v_sb[:, j0 // 128, lh, :],
                            exs[jt][:, lh, :],
                            start=(jt == 0), stop=(jt == NJT - 1))

                for r in range(r0, r1):
                    scores_exp(2 * r)
                    scores_exp(2 * r + 1)
                    if r >= 3:
                        attnv(2 * r - 6)
                        attnv(2 * r - 5)
                    if interleave and (pop_every or r % 2 == 1):
                        interleave.pop(0)()
                if r1 < NJT // 2:
                    return
                for jt in range(NJT - 6, NJT):
                    attnv(jt)
                if interleave:
                    interleave.pop(0)()
                # --- normalize + bounce (recip from PSUM, PE broadcast) ---
                # den >= O(100) always, so the fast approx (18 bits) is
                # safe and ~5x cheaper than the exact single-lane recip.
                # The bitwise seed path needs SBUF input: copy out of PSUM.
                dsb = workp.tile([1, LH, TCH], F32, tag="dsb", bufs=4,
                                 name=f"dsb{c}")
                for lh in range(LH):
                    nc.vector.tensor_copy(dsb[:, lh, :],
                                          pos_tiles[c][lh][C:C + 1, :])
                recd = workp.tile([1, LH, TCH], F32, tag="recd", bufs=4,
                                  name=f"recd{c}")
                for lh in range(LH):
                    nc.vector.reciprocal_approx_fast(
                        out=recd[:, lh, :], in_=dsb[:, lh, :])
                recb = workp.tile([1, LH, TCH], BF16, tag="recb", bufs=4,
                                  name=f"recb{c}")
                nc.vector.tensor_copy(recb, recd)
                for lh in range(LH):
                    dnb = ps1p.tile([C, TCH], F32, tag="ps1",
                                    name=f"dnb{c}_{lh}")
                    nc.tensor.matmul(dnb, bsel[0:1, 0:64],
                                     recb[:, lh, :],
                                     start=True, stop=True)
                    posb = workp.tile([C, TCH], BF16, tag="posb",
                                      name=f"posb{c}_{lh}")
                    nc.vector.tensor_copy(posb, pos_tiles[c][lh][0:C, :])
                    attbf = workp.tile([C, TCH], BF16, tag="attbf",
                                       name=f"attbf{c}_{lh}")
                    nc.vector.tensor_mul(attbf, posb, dnb)
                    # rows d*128 + 64*lh + p; chunk parity picks the
                    # 64-token half of each dest's 128-token pair block
                    nc.gpsimd.dma_start(
                        out=bass.AP(
                            tensor=bins[c // 2].tensor,
                            offset=bins[c // 2].offset + C * lh * 128
                            + (c % 2) * 64,
                            ap=[[128, C], [128 * 128, 8], [1, 64]]),
                        in_=attbf.rearrange("p (d t) -> p d t", d=8))

            def collective(q):
                nc.gpsimd.collective_compute(
                    "AllToAll", mybir.AluOpType.bypass,
                    replica_groups=[list(range(NCORES))],
                    ins=[bins[q][:, :].opt()],
                    outs=[bouts[q][:, :].opt()])

            # =============== phase-3 (per eighth-pair) ===================
            def p3_eighth(q):
                atta = workp.tile([128, 8, 128], BF16, tag="atta",
                                  name=f"atta{q}")
                nc.scalar.dma_start(
                    out=atta,
                    in_=bouts[q].rearrange("(g p) t -> p g t", p=128))
                for nh in range(2):
                    po3 = ps1p.tile([128, 512], F32, tag="ps1",
                                    name=f"po3_{q}_{nh}")
                    for a in range(8):
                        nc.tensor.matmul(
                            po3, atta[:, a, :],
                            wo_sb[:, a, nh * 512:(nh + 1) * 512],
                            start=(a == 0), stop=(a == 7))
                    outsb = workp.tile([128, 512], F32, tag="outsb",
                                       name=f"outsb{q}_{nh}")
                    nc.vector.tensor_copy(outsb, po3)
                    nc.sync.dma_start(
                        out=out[q * 128:(q + 1) * 128,
                                nh * 512:(nh + 1) * 512],
                        in_=outsb)

            # =============== schedule ====================================
            # full-size warmup AllToAll: absorbs the one-time collective
            # setup for the 128KB shape while p1 runs (the trigger does
            # not block the gpsimd queue; only collective order matters).
            nc.gpsimd.dma_start(out=wbin.rearrange("(g p) t -> p g t",
                                                   p=128),
                                in_=wseed)
            nc.gpsimd.collective_compute(
                "AllToAll", mybir.AluOpType.bypass,
                replica_groups=[list(range(NCORES))],
                ins=[wbin[:, :].opt()], outs=[wbout[:, :].opt()])

            # xt0/xt2 head the sync ring (xt1 alone on scalar) so the
            # first projections start as early as possible; the rope
            # halves queue behind them on the same ring.
            load_xt(0)
            load_xt(1)
            load_xt(2)
            nc.sync.dma_start(out=rope_sb[:, :, 0:S // 2],
                              in_=rope4[:, :, 0:S // 2])
            nc.sync.dma_start(out=rope_sb[:, :, S // 2:S],
                              in_=rope4[:, :, S // 2:S])

            for pair in (0, 2):
                for ch in (pair, pair + 1):
                    p1_q(ch)
                    p1_k(ch)
                    p1_v(ch)
                    p1_ms(ch)
                    p1_rstd(ch)
                p1_fin(pair)
                if pair == 0:
                    # WAW-dependency on rstd(0) keeps the 2MB wo load out
                    # of the startup HBM burst (it is needed only by p3)
                    nc.gpsimd.tensor_copy(wo_sb[0:1, 0:1, 0:1],
                                          st[0]["rstd"][0:1, 0:1])
                    nc.gpsimd.dma_start(out=wo_sb, in_=wo4)
                p1_fin(pair + 1)

            ilq = []
            for pair in (4, 6):
                for ch in (pair, pair + 1):
                    ilq.append(lambda ch=ch: p1_q(ch))
                    ilq.append(lambda ch=ch: p1_k(ch))
                    ilq.append(lambda ch=ch: p1_v(ch))

                    def ms_rstd(ch=ch):
                        p1_ms(ch)
                        p1_rstd(ch)
                    ilq.append(ms_rstd)
                ilq.append(lambda ch=pair: p1_fin(ch))
                ilq.append(lambda ch=pair + 1: p1_fin(ch))

            for c in range(NCH):
                p2_part(c, 0, NJT // 2, ilq, pop_every=(c < 4))
                if c % 2 == 1:
                    collective(c // 2)
                    if c >= 3:
                        ilq.append(lambda k=c // 2 - 1: p3_eighth(k))
            while ilq:
                ilq.pop(0)()
            p3_eighth(NPAIR - 1)

    nc.compile()
    return nc


def kernel(x, rope_emb, Wq, Wk, Wv, q_norm_w, k_norm_w, Wout):
    global LAST_RESULTS
    if "nc" not in _CACHE:
        _CACHE["nc"] = _build()
    nc = _CACHE["nc"]
    bf16 = ml_dtypes.bfloat16

    # batch-major tokens: t = b*S + s
    x2 = np.ascontiguousarray(
        np.transpose(np.asarray(x, np.float32), (1, 0, 2)).reshape(T, D))
    xT_np = np.ascontiguousarray(x2.T.astype(bf16))

    re = np.asarray(rope_emb, np.float32)
    cosT = np.ascontiguousarray(re[:, :, 0, 0].T)    # [32, S]
    r01T = np.ascontiguousarray(re[:, :, 0, 1].T)
    r10T = np.ascontiguousarray(re[:, :, 1, 0].T)
    ropeA_np = np.concatenate([cosT, cosT, cosT, cosT], axis=0)
    ropeB_np = np.concatenate([r01T, r10T, r01T, r10T], axis=0)
    ropeAB_np = np.ascontiguousarray(
        np.concatenate([ropeA_np[:, None, :], ropeB_np[:, None, :]],
                       axis=1).reshape(LC, 2 * S).astype(bf16))

    qw_np = np.asarray(q_norm_w, np.float32)
    kw_np = np.asarray(k_norm_w, np.float32)
    Wq_s = np.asarray(Wq, np.float32) * np.tile(qw_np, H)[None, :]
    Wk_s = np.asarray(Wk, np.float32) * np.tile(kw_np, H)[None, :]
    Wv = np.asarray(Wv, np.float32)
    Wout = np.ascontiguousarray(np.asarray(Wout, np.float32).astype(bf16))

    # cols 0:4 = q-pass selector (k rows zero), cols 4:8 = k-pass selector
    winv_np = np.zeros((128, 8), np.float32)
    winv_np[0:64, 0] = 1.0 / (qw_np * qw_np)
    winv_np[64:128, 1] = 1.0 / (qw_np * qw_np)
    winv_np[0:64, 6] = 1.0 / (kw_np * kw_np)
    winv_np[64:128, 7] = 1.0 / (kw_np * kw_np)
    winv_np = winv_np.astype(bf16)

    # rope pair-swap permutation: rot[m] = raw[sigma(m)], sigma swaps 32-row
    # halves within each 64-row head group; perm[kp, m] = 1 iff kp=sigma(m)
    perm_np = np.zeros((128, 128), np.float32)
    for m in range(128):
        g = (m // 64) * 64
        r = m - g
        sig = g + (r + 32) % 64
        perm_np[sig, m] = 1.0
    perm_np = perm_np.astype(bf16)

    # broadcast selector matrices (see kernel: bq/bk/dnb PE broadcasts)
    bsel_np = np.zeros((4, 256), np.float32)
    bsel_np[0, 0:64] = 1.0
    bsel_np[1, 64:128] = 1.0
    bsel_np[2, 128:192] = 1.0
    bsel_np[3, 192:256] = 1.0
    bsel_np = bsel_np.astype(bf16)

    in_maps = []
    for g in range(NCORES):
        sl = slice(g * LC, (g + 1) * LC)
        wqkv_np = np.ascontiguousarray(np.concatenate(
            [Wq_s[:, sl], Wk_s[:, sl], Wv[:, sl]], axis=1).astype(bf16))
        in_maps.append({
            "xT": xT_np,
            "wqkv": wqkv_np,
            "wo": Wout,
            "ropeAB": ropeAB_np,
            "winv": winv_np,
            "perm": perm_np,
            "bsel": bsel_np,
        })

    res = run_bass_kernel_spmd(nc, in_maps, core_ids=list(range(NCORES)))
    LAST_RESULTS = res
    # core g, pair k rows [k*128, k*128+64) = chunk 2k's slice g,
    # rows [k*128+64, k*128+128) = chunk 2k+1's slice g
    out_full = np.empty((T, D), np.float32)
    for g in range(NCORES):
        og = res.results[g]["out"]
        for k in range(NCH // 2):
            out_full[(16 * k + g) * 64:(16 * k + g + 1) * 64] = \
                og[k * 128:k * 128 + 64]
            out_full[(16 * k + 8 + g) * 64:(16 * k + 8 + g + 1) * 64] = \
                og[k * 128 + 64:k * 128 + 128]
    return np.ascontiguousarray(
        out_full.reshape(B, S, D).transpose(1, 0, 2))


# revision 39
# speedup vs baseline: 1.2822x; 1.2822x over previous
"""Trainium2 Bass kernel for nn_Attention (S=2048, B=2, D=1024, H=16, C=64).

Tensor-parallel over heads across 8 NeuronCores (2 heads/core), fully
interleaved wavefront:
  - All static inputs are pre-cast to bf16 on the host, so every load is
    a plain (non-casting) DMA and queue assignment is free.
  - p1 (projections+norm+rope, 8 chunks of 512 tokens): Wq/Wk pre-scaled
    by the RMSNorm weights on host; sumsq of the raw projection recovered
    via matmul against 1/w^2 selector columns; rstd computed as
    exp(-0.5*ln(ms/C+eps)) on ACT -- Ln and Exp share one activation
    table with the softmax Exp, so the kernel never swaps ACT tables;
    rstd returns as a stride-0 broadcast DMA and is applied as the last
    multiply (rope commutes with the per-token scale); the rope
    pair-swap is a PE permutation matmul.  The elementwise chain is bf16
    and split DVE/GpSimd: chunks 0-3 run fully on DVE (GpSimd is kept
    clear so the warmup collective can block it harmlessly), chunks 4-7
    use GpSimd for the adds/final muls.
  - p2 (attention, 8 query chunks of 512): scores transposed [keys, q]
    per head with K=64 contraction on PE quadrants; one [128,1024] exp
    per key block covers both heads; attn@v accumulates in PSUM with an
    appended ones column so the softmax denominator falls out.  The
    denominator reciprocal runs on DVE straight out of PSUM and returns
    as a stride-0 broadcast DMA (sync+vector rings), no reshape hops.
  - The AllToAll re-shard runs at chunk granularity (8 collectives,
    destination core = (token//64) % 8), each fired right after its
    chunk.  All collectives issue from GpSimd (NRT needs straight-line
    collective order); a FULL-SIZE warmup AllToAll fires at kernel start
    so the ~70us one-time setup for the 128KB transfer shape overlaps
    p1 instead of stalling the first real collective.  p3 out-projection
    runs per received eighth with its DMAs on the vector ring so a slow
    collective cannot head-of-line-block the sync ring.
  - p1 pieces and p3 eighths drain from an interleave queue between p2
    key blocks so the PE stays continuously busy.
"""

import sys

if "/opt/trn_rl_repo" not in sys.path:
    sys.path.insert(0, "/opt/trn_rl_repo")

import numpy as np
import ml_dtypes
import concourse.bass as bass
import concourse.hw_specs as _hw_specs
from concourse import bacc, tile, mybir
from concourse.bass_utils import run_bass_kernel_spmd
from concourse.masks import make_identity

# The act-table selector is first-fit per function, which lands Exp in
# 'exp_and_others' and Ln in 'natural_log' and then thrashes 1.3us table
# loads between them.  'natural_log_exp_and_others' genuinely contains
# every ACT function this kernel uses (Exp, Ln, Square), so mask those
# functions out of the other tables; the emitted act_func_set_id then
# points at the real combined table in act_info.json.
_ORIG_ACT_TABLES = _hw_specs.get_activation_tables


def _combined_act_tables(arch):
    AFT = mybir.ActivationFunctionType
    keep = {AFT.Exp, AFT.Ln, AFT.Square}
    out = {}
    for name, funcs in _ORIG_ACT_TABLES(arch).items():
        if name != "natural_log_exp_and_others":
            funcs = set(funcs) - keep
        out[name] = set(funcs)
    return out


bacc.get_activation_tables = _combined_act_tables

S, B, D, H, C = 2048, 2, 1024, 16, 64
EPS = 1e-6
NCORES = 8
T = S * B                  # 4096 tokens, batch-major: t = b*S + s
LH = H // NCORES           # 2 local heads
LC = LH * C                # 128 local head columns
TCH = 512                  # p1/p2 token chunk
NCH = T // TCH             # 8
NJT = S // 128             # 16 key blocks per batch
TOK_OUT = T // NCORES      # 512 output tokens per core

F32 = mybir.dt.float32
F32R = mybir.dt.float32r
BF16 = mybir.dt.bfloat16
AF = mybir.ActivationFunctionType

_CACHE = {}
LAST_RESULTS = None


def _build():
    nc = bacc.Bacc("TRN2", target_bir_lowering=False, debug=False,
                   num_devices=NCORES)
    xT = nc.dram_tensor("xT", [D, T], BF16, kind="ExternalInput")
    wqkv = nc.dram_tensor("wqkv", [D, 3 * LC], BF16, kind="ExternalInput")
    wo = nc.dram_tensor("wo", [H * C, D], BF16, kind="ExternalInput")
    ropeAB = nc.dram_tensor("ropeAB", [LC, 2 * S], BF16,
                            kind="ExternalInput")
    winv = nc.dram_tensor("winv", [128, 8], BF16, kind="ExternalInput")
    perm = nc.dram_tensor("perm", [128, 128], BF16, kind="ExternalInput")
    bseld = nc.dram_tensor("bsel", [4, 256], BF16, kind="ExternalInput")
    out = nc.dram_tensor("out", [TOK_OUT, D], F32, kind="ExternalOutput")

    xT4 = xT.rearrange("(a p) t -> p a t", p=128)        # [128, 8, T]
    wqkv4 = wqkv.rearrange("(a p) c -> p a c", p=128)    # [128, 8, 3*LC]
    wo4 = wo.rearrange("(a p) n -> p a n", p=128)        # [128, 8, D]
    rope4 = ropeAB.rearrange("p (j t) -> p j t", j=2)    # [128, 2, S]

    with tile.TileContext(nc) as tc:
        with (
            tc.tile_pool(name="singles", bufs=1) as singles,
            tc.tile_pool(name="xtp", bufs=5) as xtp,
            tc.tile_pool(name="workp", bufs=2) as workp,
            tc.tile_pool(name="ps1", bufs=2, space="PSUM") as ps1p,
            tc.tile_pool(name="pss", bufs=2, space="PSUM") as pssp,
            tc.tile_pool(name="pos", bufs=2, space="PSUM") as posp,
            tc.tile_pool(name="dram", bufs=1, space="DRAM") as dram,
        ):
            # ---- constants ----
            ident = singles.tile([128, 128], BF16)
            make_identity(nc, ident)
            winv_sb = singles.tile([128, 8], BF16)
            nc.gpsimd.dma_start(out=winv_sb, in_=winv[:, :])
            permb = singles.tile([128, 128], BF16)
            nc.gpsimd.dma_start(out=permb, in_=perm[:, :])
            eps128 = singles.tile([128, 1], F32)
            nc.vector.memset(eps128, EPS)
            wseed = singles.tile([128, 8, 64], BF16)
            nc.vector.memset(wseed, 0.0)
            # broadcast selectors: bq rows <- rstd rows 0/1, bk <- 2/3,
            # dnb <- row 0 of its operand (bsel[0:1, 0:64] is all-ones)
            bsel = singles.tile([4, 256], BF16)
            nc.gpsimd.dma_start(out=bsel, in_=bseld[:, :])

            # ---- weights / rope (wo deferred: only p3 needs it) ----
            wqkv_sb = singles.tile([128, 8, 3 * LC], BF16)
            nc.sync.dma_start(out=wqkv_sb, in_=wqkv4)
            rope_sb = singles.tile([128, 2, S], BF16)
            wo_sb = singles.tile([128, 8, D], BF16)

            # ---- persistent activations ----
            qT_sb = singles.tile([128, T], BF16)   # [2 heads x 64c, t]
            k_sb = singles.tile([128, T], BF16)
            v_sb = singles.tile([128, T // 128, LH, C + 1], BF16)
            onescol = singles.tile([128, T // 128, LH, 1], F32)
            nc.vector.memset(onescol, 1.0)
            nc.vector.tensor_copy(v_sb[:, :, :, C:C + 1], onescol)

            NPAIR = NCH // 2
            wbin = dram.tile([NCORES * 128, 64], BF16, tag="wbin",
                             name="wbin")
            wbout = dram.tile([NCORES * 128, 64], BF16, tag="wbout",
                              name="wbout")
            bins = [dram.tile([NCORES * 128, 64], BF16, tag=f"bin{q}",
                              name=f"bin{q}") for q in range(NCH)]
            bouts = [dram.tile([NCORES * 128, 64], BF16, tag=f"bout{q}",
                               name=f"bout{q}") for q in range(NCH)]

            xts = {}
            st = {ch: {} for ch in range(NCH)}

            def load_xt(ch):
                xt = xtp.tile([128, 8, TCH], BF16, tag="xt", name=f"xt{ch}")
                eng = nc.sync if ch % 2 == 0 else nc.scalar
                sl = slice(ch * TCH, (ch + 1) * TCH)
                if ch < 2:
                    # split: the a<4 half lands first so the first
                    # projection matmuls start ~8us earlier
                    eng.dma_start(out=xt[:, 0:4, :], in_=xT4[:, 0:4, sl])
                    eng.dma_start(out=xt[:, 4:8, :], in_=xT4[:, 4:8, sl])
                else:
                    eng.dma_start(out=xt, in_=xT4[:, :, sl])
                xts[ch] = xt

            # ---- p1 pieces (ACT: squares for ch<4, ln+exp rstd) ----
            def p1_q(ch):
                if ch + 3 < NCH:
                    load_xt(ch + 3)
                psq = ps1p.tile([128, TCH], F32, tag="ps1", name=f"psq{ch}")
                for a in range(8):
                    nc.tensor.matmul(psq, wqkv_sb[:, a, 0:LC],
                                     xts[ch][:, a, :],
                                     start=(a == 0), stop=(a == 7))
                qraw = workp.tile([128, TCH], BF16, tag="qraw", bufs=6,
                                  name=f"qraw{ch}")
                nc.vector.tensor_copy(qraw, psq)
                sq2 = workp.tile([128, TCH], BF16, tag="sq2",
                                 name=f"sq2_{ch}")
                if ch < 4:
                    nc.scalar.activation(sq2, psq, AF.Square)
                else:
                    nc.vector.tensor_mul(sq2, qraw, qraw)
                st[ch]["qraw"], st[ch]["sq2"] = qraw, sq2

            def p1_k(ch):
                psk = ps1p.tile([128, TCH], F32, tag="ps1", name=f"psk{ch}")
                for a in range(8):
                    nc.tensor.matmul(psk, wqkv_sb[:, a, LC:2 * LC],
                                     xts[ch][:, a, :],
                                     start=(a == 0), stop=(a == 7))
                kraw = workp.tile([128, TCH], BF16, tag="kraw", bufs=6,
                                  name=f"kraw{ch}")
                nc.vector.tensor_copy(kraw, psk)
                sqk2 = workp.tile([128, TCH], BF16, tag="sqk2",
                                  name=f"sqk2_{ch}")
                if ch < 4:
                    nc.scalar.activation(sqk2, psk, AF.Square)
                else:
                    nc.vector.tensor_mul(sqk2, kraw, kraw)
                st[ch]["kraw"], st[ch]["sqk2"] = kraw, sqk2

            def p1_v(ch):
                t0 = ch * TCH
                psv = ps1p.tile([128, TCH], F32, tag="ps1", name=f"psv{ch}")
                for a in range(8):
                    nc.tensor.matmul(psv, wqkv_sb[:, a, 2 * LC:3 * LC],
                                     xts[ch][:, a, :],
                                     start=(a == 0), stop=(a == 7))
                vt = workp.tile([128, TCH], BF16, tag="vt", name=f"vt{ch}")
                nc.vector.tensor_copy(vt, psv)
                ptv = ps1p.tile([128, 4, 128], BF16, tag="ps1",
                                name=f"ptv{ch}")
                for s5 in range(TCH // 128):
                    nc.tensor.transpose(ptv[:, s5, :],
                                        vt[:, s5 * 128:(s5 + 1) * 128],
                                        ident)
                blk0 = t0 // 128
                nc.vector.tensor_copy(
                    v_sb[:, blk0:blk0 + 4, :, 0:C],
                    ptv.rearrange("p f (l c) -> p f l c", l=LH))

            def p1_ms(ch):
                # sumsq rows 0:2 = q heads, 2:4 = k heads (one bank)
                ms = ps1p.tile([4, TCH], F32, tag="ps1", name=f"ms{ch}")
                nc.tensor.matmul(ms, winv_sb[:, 0:4], st[ch]["sq2"],
                                 start=True, stop=False)
                nc.tensor.matmul(ms, winv_sb[:, 4:8], st[ch]["sqk2"],
                                 start=False, stop=True)
                st[ch]["ms"] = ms

            def p1_rstd(ch):
                # rstd = exp(-0.5 * ln(ms/C + eps)); Ln+Exp live in the
                # same ACT table as the softmax Exp -> no table swaps.
                lnv = workp.tile([4, TCH], F32, tag="lnv", bufs=4,
                                 name=f"lnv{ch}")
                nc.scalar.activation(lnv, st[ch]["ms"], AF.Ln,
                                     bias=eps128[0:4, :], scale=1.0 / C)
                rstd = workp.tile([4, TCH], BF16, tag="rstd", bufs=4,
                                  name=f"rstd{ch}")
                nc.scalar.activation(rstd, lnv, AF.Exp, bias=0.0,
                                     scale=-0.5)
                st[ch]["rstd"] = rstd

            def p1_fin(ch):
                t0 = ch * TCH
                t0s = t0 % S
                # per-token rstd broadcast via K=4 selector matmuls (bf16)
                for which, raw, c0, dst in (
                        ("q", st[ch]["qraw"], 0, qT_sb),
                        ("k", st[ch]["kraw"], 128, k_sb)):
                    bw = ps1p.tile([128, TCH], F32, tag="ps1",
                                   name=f"bw{which}{ch}")
                    nc.tensor.matmul(bw, bsel[:, c0:c0 + 128],
                                     st[ch]["rstd"], start=True, stop=True)
                    t1 = workp.tile([128, TCH], BF16, tag="t1",
                                    name=f"t1{which}{ch}")
                    nc.vector.tensor_mul(t1, rope_sb[:, 0, t0s:t0s + TCH],
                                         raw)
                    rot = ps1p.tile([128, TCH], F32, tag="ps1",
                                    name=f"rot{which}{ch}")
                    nc.tensor.matmul(rot, permb, raw, start=True, stop=True)
                    t2 = workp.tile([128, TCH], BF16, tag="t2",
                                    name=f"t2{which}{ch}")
                    nc.vector.tensor_mul(t2, rope_sb[:, 1, t0s:t0s + TCH],
                                         rot)
                    t3 = workp.tile([128, TCH], BF16, tag="t3",
                                    name=f"t3{which}{ch}")
                    nc.gpsimd.tensor_add(t3, t1, t2)
                    nc.vector.tensor_mul(dst[:, t0:t0 + TCH], t3, bw)

            # =============== phase-2 =====================================
            pos_tiles = {}

            p2_exs = {}

            def p2_part(c, r0, r1, interleave, pop_every=False):
                b = c // 4
                q0 = c * TCH
                if c not in pos_tiles:
                    pos_tiles[c] = [posp.tile([C + 1, TCH], F32, tag="pos",
                                              name=f"pos{c}_{lh}")
                                    for lh in range(LH)]
                    p2_exs[c] = {}
                exs = p2_exs[c]

                def scores_exp(jt):
                    j0 = b * S + jt * 128
                    pss = pssp.tile([128, LH, TCH], F32, tag="pss",
                                    name=f"pss{c}_{jt}")
                    for lh in range(LH):
                        nc.tensor.matmul(
                            pss[:, lh, :],
                            k_sb[64 * lh:64 * lh + 64, j0:j0 + 128],
                            qT_sb[64 * lh:64 * lh + 64, q0:q0 + TCH],
                            start=True, stop=True)
                    ex = workp.tile([128, LH, TCH], BF16, tag="ex", bufs=11,
                                    name=f"ex{c}_{jt}")
                    nc.scalar.activation(ex, pss, AF.Exp, bias=0.0,
                                         scale=0.125)
                    exs[jt] = ex

                def attnv(jt):
                    j0 = b * S + jt * 128
                    for lh in range(LH):
                        nc.tensor.matmul(
                            pos_tiles[c][lh],
                            v_sb[:, j0 // 128, lh, :],
                            exs[jt][:, lh, :],
                            start=(jt == 0), stop=(jt == NJT - 1))

                for r in range(r0, r1):
                    scores_exp(2 * r)
                    scores_exp(2 * r + 1)
                    if r >= 3:
                        attnv(2 * r - 6)
                        attnv(2 * r - 5)
                    if interleave and (pop_every or r % 2 == 1):
                        interleave.pop(0)()
                if r1 < NJT // 2:
                    return
                for jt in range(NJT - 6, NJT):
                    attnv(jt)
                if interleave:
                    interleave.pop(0)()
                # --- normalize + bounce (recip from PSUM, PE broadcast) ---
                # den >= O(100) always, so the fast approx (18 bits) is
                # safe and ~5x cheaper than the exact single-lane recip.
                # The bitwise seed path needs SBUF input: copy out of PSUM.
                dsb = workp.tile([1, LH, TCH], F32, tag="dsb", bufs=4,
                                 name=f"dsb{c}")
                for lh in range(LH):
                    nc.vector.tensor_copy(dsb[:, lh, :],
                                          pos_tiles[c][lh][C:C + 1, :])
                recd = workp.tile([1, LH, TCH], F32, tag="recd", bufs=4,
                                  name=f"recd{c}")
                for lh in range(LH):
                    nc.vector.reciprocal_approx_fast(
                        out=recd[:, lh, :], in_=dsb[:, lh, :])
                recb = workp.tile([1, LH, TCH], BF16, tag="recb", bufs=4,
                                  name=f"recb{c}")
                nc.vector.tensor_copy(recb, recd)
                for lh in range(LH):
                    dnb = ps1p.tile([C, TCH], F32, tag="ps1",
                                    name=f"dnb{c}_{lh}")
                    nc.tensor.matmul(dnb, bsel[0:1, 0:64],
                                     recb[:, lh, :],
                                     start=True, stop=True)
                    posb = workp.tile([C, TCH], BF16, tag="posb",
                                      name=f"posb{c}_{lh}")
                    nc.vector.tensor_copy(posb, pos_tiles[c][lh][0:C, :])
                    attbf = workp.tile([C, TCH], BF16, tag="attbf",
                                       name=f"attbf{c}_{lh}")
                    nc.vector.tensor_mul(attbf, posb, dnb)
                    # rows d*128 + 64*lh + p, 64-token slices per dest
                    nc.gpsimd.dma_start(
                        out=bass.AP(
                            tensor=bins[c].tensor,
                            offset=bins[c].offset + C * lh * 64,
                            ap=[[64, C], [128 * 64, 8], [1, 64]]),
                        in_=attbf.rearrange("p (d t) -> p d t", d=8))

            def collective(q):
                nc.gpsimd.collective_compute(
                    "AllToAll", mybir.AluOpType.bypass,
                    replica_groups=[list(range(NCORES))],
                    ins=[bins[q][:, :].opt()],
                    outs=[bouts[q][:, :].opt()])

            # =============== phase-3 (per eighth-pair) ===================
            # consumes bouts[2q] and bouts[2q+1] side by side so the PE
            # runs full 128-partition outputs (half the p3 columns)
            def p3_eighth(q):
                atta = workp.tile([128, 8, 128], BF16, tag="atta",
                                  name=f"atta{q}")
                for h in range(2):
                    nc.scalar.dma_start(
                        out=atta[:, :, h * 64:(h + 1) * 64],
                        in_=bouts[2 * q + h].rearrange("(g p) t -> p g t",
                                                       p=128))
                for nh in range(2):
                    po3 = ps1p.tile([128, 512], F32, tag="ps1",
                                    name=f"po3_{q}_{nh}")
                    for a in range(8):
                        nc.tensor.matmul(
                            po3, atta[:, a, :],
                            wo_sb[:, a, nh * 512:(nh + 1) * 512],
                            start=(a == 0), stop=(a == 7))
                    outsb = workp.tile([128, 512], F32, tag="outsb",
                                       name=f"outsb{q}_{nh}")
                    nc.vector.tensor_copy(outsb, po3)
                    nc.sync.dma_start(
                        out=out[q * 128:(q + 1) * 128,
                                nh * 512:(nh + 1) * 512],
                        in_=outsb)

            # =============== schedule ====================================
            # full-size warmup AllToAll: absorbs the one-time collective
            # setup for the 128KB shape while p1 runs (the trigger does
            # not block the gpsimd queue; only collective order matters).
            nc.gpsimd.dma_start(out=wbin.rearrange("(g p) t -> p g t",
                                                   p=128),
                                in_=wseed)
            nc.gpsimd.collective_compute(
                "AllToAll", mybir.AluOpType.bypass,
                replica_groups=[list(range(NCORES))],
                ins=[wbin[:, :].opt()], outs=[wbout[:, :].opt()])

            # xt0/xt2 head the sync ring (xt1 alone on scalar) so the
            # first projections start as early as possible; the rope
            # halves queue behind them on the same ring.
            load_xt(0)
            load_xt(1)
            load_xt(2)
            nc.sync.dma_start(out=rope_sb[:, :, 0:S // 2],
                              in_=rope4[:, :, 0:S // 2])
            nc.sync.dma_start(out=rope_sb[:, :, S // 2:S],
                              in_=rope4[:, :, S // 2:S])

            for pair in (0, 2):
                for ch in (pair, pair + 1):
                    p1_q(ch)
                    p1_k(ch)
                    p1_v(ch)
                    p1_ms(ch)
                    p1_rstd(ch)
                p1_fin(pair)
                if pair == 0:
                    # WAW-dependency on rstd(0) keeps the 2MB wo load out
                    # of the startup HBM burst (it is needed only by p3)
                    nc.gpsimd.tensor_copy(wo_sb[0:1, 0:1, 0:1],
                                          st[0]["rstd"][0:1, 0:1])
                    nc.gpsimd.dma_start(out=wo_sb, in_=wo4)
                p1_fin(pair + 1)

            ilq = []
            for pair in (4, 6):
                for ch in (pair, pair + 1):
                    ilq.append(lambda ch=ch: p1_q(ch))
                    ilq.append(lambda ch=ch: p1_k(ch))
                    ilq.append(lambda ch=ch: p1_v(ch))

                    def ms_rstd(ch=ch):
                        p1_ms(ch)
                        p1_rstd(ch)
                    ilq.append(ms_rstd)
                ilq.append(lambda ch=pair: p1_fin(ch))
                ilq.append(lambda ch=pair + 1: p1_fin(ch))

            for c in range(NCH):
                p2_part(c, 0, NJT // 2, ilq, pop_every=(c < 4))
                collective(c)
                if c >= 3 and c % 2 == 1:
                    ilq.append(lambda k=(c - 3) // 2: p3_eighth(k))
            while ilq:
                ilq.pop(0)()
            p3_eighth(NPAIR - 1)

    nc.compile()
    return nc


def kernel(x, rope_emb, Wq, Wk, Wv, q_norm_w, k_norm_w, Wout):
    global LAST_RESULTS
    if "nc" not in _CACHE:
        _CACHE["nc"] = _build()
    nc = _CACHE["nc"]
    bf16 = ml_dtypes.bfloat16

    # batch-major tokens: t = b*S + s
    x2 = np.ascontiguousarray(
        np.transpose(np.asarray(x, np.float32), (1, 0, 2)).reshape(T, D))
    xT_np = np.ascontiguousarray(x2.T.astype(bf16))

    re = np.asarray(rope_emb, np.float32)
    cosT = np.ascontiguousarray(re[:, :, 0, 0].T)    # [32, S]
    r01T = np.ascontiguousarray(re[:, :, 0, 1].T)
    r10T = np.ascontiguousarray(re[:, :, 1, 0].T)
    ropeA_np = np.concatenate([cosT, cosT, cosT, cosT], axis=0)
    ropeB_np = np.concatenate([r01T, r10T, r01T, r10T], axis=0)
    ropeAB_np = np.ascontiguousarray(
        np.concatenate([ropeA_np[:, None, :], ropeB_np[:, None, :]],
                       axis=1).reshape(LC, 2 * S).astype(bf16))

    qw_np = np.asarray(q_norm_w, np.float32)
    kw_np = np.asarray(k_norm_w, np.float32)
    Wq_s = np.asarray(Wq, np.float32) * np.tile(qw_np, H)[None, :]
    Wk_s = np.asarray(Wk, np.float32) * np.tile(kw_np, H)[None, :]
    Wv = np.asarray(Wv, np.float32)
    Wout = np.ascontiguousarray(np.asarray(Wout, np.float32).astype(bf16))

    # cols 0:4 = q-pass selector (k rows zero), cols 4:8 = k-pass selector
    winv_np = np.zeros((128, 8), np.float32)
    winv_np[0:64, 0] = 1.0 / (qw_np * qw_np)
    winv_np[64:128, 1] = 1.0 / (qw_np * qw_np)
    winv_np[0:64, 6] = 1.0 / (kw_np * kw_np)
    winv_np[64:128, 7] = 1.0 / (kw_np * kw_np)
    winv_np = winv_np.astype(bf16)

    # rope pair-swap permutation: rot[m] = raw[sigma(m)], sigma swaps 32-row
    # halves within each 64-row head group; perm[kp, m] = 1 iff kp=sigma(m)
    perm_np = np.zeros((128, 128), np.float32)
    for m in range(128):
        g = (m // 64) * 64
        r = m - g
        sig = g + (r + 32) % 64
        perm_np[sig, m] = 1.0
    perm_np = perm_np.astype(bf16)

    # broadcast selector matrices (see kernel: bq/bk/dnb PE broadcasts)
    bsel_np = np.zeros((4, 256), np.float32)
    bsel_np[0, 0:64] = 1.0
    bsel_np[1, 64:128] = 1.0
    bsel_np[2, 128:192] = 1.0
    bsel_np[3, 192:256] = 1.0
    bsel_np = bsel_np.astype(bf16)

    in_maps = []
    for g in range(NCORES):
        sl = slice(g * LC, (g + 1) * LC)
        wqkv_np = np.ascontiguousarray(np.concatenate(
            [Wq_s[:, sl], Wk_s[:, sl], Wv[:, sl]], axis=1).astype(bf16))
        in_maps.append({
            "xT": xT_np,
            "wqkv": wqkv_np,
            "wo": Wout,
            "ropeAB": ropeAB_np,
            "winv": winv_np,
            "perm": perm_np,
            "bsel": bsel_np,
        })

    res = run_bass_kernel_spmd(nc, in_maps, core_ids=list(range(NCORES)))
    LAST_RESULTS = res
    # core g, pair k rows [k*128, k*128+64) = chunk 2k's slice g,
    # rows [k*128+64, k*128+128) = chunk 2k+1's slice g
    out_full = np.empty((T, D), np.float32)
    for g in range(NCORES):
        og = res.results[g]["out"]
        for k in range(NCH // 2):
            out_full[(16 * k + g) * 64:(16 * k + g + 1) * 64] = \
                og[k * 128:k * 128 + 64]
            out_full[(16 * k + 8 + g) * 64:(16 * k + 8 + g + 1) * 64] = \
                og[k * 128 + 64:k * 128 + 128]
    return np.ascontiguousarray(
        out_full.reshape(B, S, D).transpose(1, 0, 2))
